# revision 8
# baseline (speedup 1.0000x reference)
"""DisplaceChannel (integer displace + per-position 5x5 gaussian depthwise
conv) as a Bass/Tile kernel for 8 Trainium2 NeuronCores.

Math: the 5x5 gaussian kernel is separable and its normalizer factorizes;
the integer shift + 'same' zero-padding fold into banded 64x64 row/col
operators built host-side from the tiny (48,2) `offset`.  Per image:

    out = R1^T @ X @ R2        (R1 = y-operator, R2 = x-operator)

Device schedule (per core: 4 batches, 384 channels; data-parallel over
batch across the 8 cores; operators replicated):

  Groups g = 0..47 share one operator pair per 8 channels.  Groups are
  processed two at a time (a "unit" = group pair 2G, 2G+1) stacked on the
  partition axis, and every matmul is a full-square (0,0) tile with 128
  output partitions and matched LDWEIGHTS/stream cadence (128 rows/128
  cols), so the PE runs at its 128-results/cycle output roofline:

  pass1 (per channel c): lhsT = data [128(h,y), 128(s,x)] stationary,
        rhs = [[R1_2G, 0], [0, R1_2G+1]] zero-padded pair [128, 128]
        -> psum [128(s,x), 128(h,y')]      (zeros kill the cross terms)
  pass2 (per half h, pair m): lhsT = pass1 [128(s,x), 128(c2,y')] fp16,
        rhs = blockdiag(R2_g, R2_g) [128, 128]
        -> psum [128(c2,y'), 128(s,x')]

  PSUM->SBUF fp32->fp16 copies run [128, 1024] at a time (2 banks),
  alternating DVE / ACT.  Emission is software-pipelined (pass1 of unit
  u+1 precedes pass2 of unit u) so the in-order PE queue never stalls on
  a copy.

DMA: ALL input chunks are issued up-front into dedicated SBUF buffers
(12.6 MB resident) - batch-pair 0 on the sync HWDGE ring, pair 1 on the
scalar HWDGE ring - so the input stream runs at line rate with no
compute-coupled waits.  Output chunks alternate between the sync ring
(FIFO behind the inputs, which naturally prioritizes input) and the
otherwise-idle gpsimd SWDGE ring.  Operator tables load first on the
scalar ring.  I/O is fp16 with host-side pack/unpack (index permutation
only).
"""

import numpy as np

from concourse import bacc, mybir, tile
from concourse.bass_utils import run_bass_kernel_spmd

# problem constants (hardcoded per harness contract)
B_FULL, C, H, W = 32, 384, 64, 64
N_CORES = 8
B_LOC = B_FULL // N_CORES          # 4 batches per core
P_POS = 48                         # offset positions; C // P_POS = 8 chan/pos
GROUP = C // P_POS                 # 8 channels share one operator pair
KSZ, SIGMA, CK = 5, 0.5, 2

N_BPAIR = B_LOC // 2               # batch-pairs (2bp, 2bp+1) per core
NG = P_POS // 2                    # 24 group-pair units per bp
GCOLS = 2 * GROUP * 64             # 1024 cols per unit (c, s, x)
CHUNK_G = 3                        # units per DMA chunk ([128, 3072] fp16)
N_CHUNK = NG // CHUNK_G            # 8 chunks per bp
CHUNK_COLS = CHUNK_G * GCOLS       # 3072
XCOLS = NG * GCOLS                 # 24576 per-bp packed cols

FP16 = mybir.dt.float16
FP32 = mybir.dt.float32

_LAST_RESULT = None                # test.py introspection (profile/exec time)


def _shift_conv_matrix(sub, d):
    """[64(src), 64(out)] with R[src,out] = k[i], src = out + i - 2 - d,
    masked by conv zero-pad (0<=out+i-2<64) and shift zero-fill (0<=src<64)."""
    k = np.exp(-((np.arange(KSZ) - CK + sub) ** 2) / (2.0 * SIGMA**2))
    k = k / k.sum()
    R = np.zeros((H, H), dtype=np.float64)
    out = np.arange(H)
    for i in range(KSZ):
        t = out + i - CK            # coordinate in the shifted image
        src = t - d
        m = (t >= 0) & (t < H) & (src >= 0) & (src < H)
        R[src[m], out[m]] += k[i]
    return R


def _build_ops(offset):
    """ops1 [128, NG*128] fp16: unit block G = [[R1_2G, 0], [0, R1_2G+1]]
    (y-operators for the group pair, zero-padded to full contraction).
    ops2 [128, P_POS*128] fp16: per position blockdiag(R2_g, R2_g)."""
    off_round = np.round(offset.astype(np.float64))
    off_int = off_round.astype(np.int64)
    sub = offset.astype(np.float64) - off_round
    ops1 = np.zeros((128, NG * 128), dtype=np.float64)
    ops2 = np.zeros((128, P_POS * 128), dtype=np.float64)
    for p in range(P_POS):
        R1 = _shift_conv_matrix(sub[p, 1], off_int[p, 1])
        R2 = _shift_conv_matrix(sub[p, 0], off_int[p, 0])
        G, h = divmod(p, 2)
        ops1[64 * h:64 * h + 64, 128 * G + 64 * h:128 * G + 64 * h + 64] = R1
        ops2[0:64, 128 * p:128 * p + 64] = R2
        ops2[64:128, 128 * p + 64:128 * p + 128] = R2
    return ops1.astype(np.float16), ops2.astype(np.float16)


def _build_bass():
    nc = bacc.Bacc(
        "TRN2",
        target_bir_lowering=False,
        debug=False,
        num_devices=N_CORES,
    )
    # packed fp16 input: per bp a [128, 24576] block; partition = 64h + y
    # (h = group parity), col = G*1024 + c*128 + s*64 + x for channel
    # 8*(2G+h)+c of batch 2bp+s.
    x_in = nc.declare_dram_parameter("x", [N_BPAIR, 128, XCOLS], FP16,
                                     isOutput=False)
    ops1_in = nc.declare_dram_parameter("ops1", [128, NG * 128], FP16,
                                        isOutput=False)
    ops2_in = nc.declare_dram_parameter("ops2", [128, P_POS * 128], FP16,
                                        isOutput=False)
    # packed output: per bp [128, 24576] fp16; partition = c2*64 + y',
    # col = G*1024 + h*512 + m*128 + s*64 + x'; channel = 8*(2G+h)+2m+c2.
    y_out = nc.declare_dram_parameter("y", [N_BPAIR, 128, XCOLS], FP16,
                                      isOutput=True)

    with tile.TileContext(nc) as tc:
        with (
            tc.tile_pool(name="consts", bufs=1) as consts,
            tc.tile_pool(name="wchunk", bufs=2 * N_CHUNK) as wpool,
            tc.tile_pool(name="l2", bufs=3) as l2pool,
            tc.tile_pool(name="outs", bufs=6) as outpool,
            tc.tile_pool(name="psum1", bufs=2, space="PSUM") as psum1p,
            tc.tile_pool(name="psum2", bufs=2, space="PSUM") as psum2p,
        ):
            # operator tables via the scalar HWDGE ring, ahead of its
            # input chunks.
            t_ops1 = consts.tile([128, NG * 128], FP16)
            t_ops2 = consts.tile([128, P_POS * 128], FP16)
            nc.scalar.dma_start(out=t_ops1[:], in_=ops1_in[:])
            nc.scalar.dma_start(out=t_ops2[:], in_=ops2_in[:])

            # preload ALL input chunks into dedicated buffers, issued
            # unconditionally up-front: bp0 on the sync HWDGE ring, bp1 on
            # the scalar HWDGE ring.  No buffer-reuse waits anywhere.
            wts = {}
            for k in range(N_CHUNK):
                for bp in range(N_BPAIR):
                    eng = nc.sync if bp == 0 else nc.scalar
                    wt = wpool.tile([128, CHUNK_COLS], FP16)
                    eng.dma_start(
                        out=wt[:],
                        in_=x_in[bp][:, k * CHUNK_COLS:(k + 1) * CHUNK_COLS])
                    wts[(bp, k)] = wt

            units = [(bp, G) for bp in range(N_BPAIR) for G in range(NG)]
            state = {}
            outs_by_chunk = {}

            def emit_pass1(u):
                bp, G = units[u]
                k, go = divmod(G, CHUNK_G)
                wt = wts[(bp, k)]
                if go == 0:
                    outs_by_chunk[(bp, k)] = outpool.tile(
                        [128, CHUNK_COLS], FP16, name="outs")
                # ps1 cols ordered (c, h, y'); the copy into l2 transposes
                # the traversal to (h, c, y') so pass2's lhsT slices are
                # contiguous (walrus allows only one free dim on weights).
                ps1 = psum1p.tile([128, GROUP, 2, 64], FP32)
                for c in range(GROUP):
                    nc.tensor.matmul(
                        ps1[:, c, :, :],
                        wt[:, go * GCOLS + 128 * c:go * GCOLS + 128 * c + 128],
                        t_ops1[:, 128 * G:128 * G + 128],
                        start=True, stop=True)
                l2 = l2pool.tile([128, 1024], FP16)
                src = ps1[:].rearrange("p c h y -> p h c y")
                if u % 2 == 0:
                    nc.vector.tensor_copy(l2[:], src)
                else:
                    nc.scalar.copy(l2[:], src)
                state[u] = l2

            def emit_pass2(u):
                bp, G = units[u]
                k, go = divmod(G, CHUNK_G)
                l2 = state.pop(u)
                outs = outs_by_chunk[(bp, k)]
                ps2 = psum2p.tile([128, 1024], FP32)
                for h in (0, 1):
                    g = 2 * G + h
                    for m in range(GROUP // 2):
                        col = 512 * h + 128 * m
                        nc.tensor.matmul(
                            ps2[:, col:col + 128],
                            l2[:, col:col + 128],
                            t_ops2[:, 128 * g:128 * g + 128],
                            start=True, stop=True)
                od = outs[:, go * GCOLS:(go + 1) * GCOLS]
                if u % 2 == 0:
                    nc.scalar.copy(od, ps2[:])
                else:
                    nc.vector.tensor_copy(od, ps2[:])
                if go == CHUNK_G - 1:
                    j = bp * N_CHUNK + k
                    oeng = nc.sync if j % 2 == 0 else nc.gpsimd
                    oeng.dma_start(
                        out=y_out[bp][:, k * CHUNK_COLS:(k + 1) * CHUNK_COLS],
                        in_=outs[:])
                    del outs_by_chunk[(bp, k)]

            # software pipeline: pass1(u+1) is emitted before pass2(u) so
            # the in-order PE queue overlaps matmuls with the l2 copies.
            for u in range(len(units) + 1):
                if u < len(units):
                    emit_pass1(u)
                if u >= 1:
                    emit_pass2(u - 1)
    nc.compile()
    return nc


_NC_CACHE = None


def kernel(x: np.ndarray, offset: np.ndarray) -> np.ndarray:
    global _LAST_RESULT, _NC_CACHE
    assert x.shape == (B_FULL, C, H, W), x.shape
    ops1, ops2 = _build_ops(np.asarray(offset, dtype=np.float32))
    if _NC_CACHE is None:
        _NC_CACHE = _build_bass()
    nc = _NC_CACHE

    # host pack: fp16 cast + index permutation (see module docstring).
    x16 = np.asarray(x, dtype=np.float32).astype(np.float16)
    xv = x16.reshape(N_CORES, N_BPAIR, 2, NG, 2, GROUP, H, W)
    # [i, bp, s, G, h, c, y, x] -> [i, bp, h, y, G, c, s, x]
    xP = np.ascontiguousarray(xv.transpose(0, 1, 4, 6, 3, 5, 2, 7))
    xP = xP.reshape(N_CORES, N_BPAIR, 128, XCOLS)

    in_maps = []
    for i in range(N_CORES):
        in_maps.append({"x": xP[i], "ops1": ops1, "ops2": ops2})
    res = run_bass_kernel_spmd(nc, in_maps, list(range(N_CORES)))
    _LAST_RESULT = res

    # host unpack: partition = c2*64 + y, col = G*1024 + h*512 + m*128 +
    # s*64 + x; channel = 8*(2G+h) + 2m + c2, batch = 4i + 2bp + s.
    out = np.empty((B_FULL, C, H, W), dtype=np.float32)
    for i in range(N_CORES):
        yv = res.results[i]["y"].astype(np.float32).reshape(
            N_BPAIR, 2, H, NG, 2, GROUP // 2, 2, W)
        # [bp, c2, y, G, h, m, s, x] -> [bp, s, G, h, m, c2, y, x]
        yt = yv.transpose(0, 6, 3, 4, 5, 1, 2, 7)
        out[4 * i:4 * i + 4] = yt.reshape(B_LOC, C, H, W)
    return out


if __name__ == "__main__":
    nc = _build_bass()
    print("bass program built ok")
